# revision 28
# baseline (speedup 1.0000x reference)
"""MLA (multi-head latent attention) Bass kernel for Trainium2, 8 NeuronCores.

Sharding: pure data-parallel over batch (B=8 -> one batch element per core).
Each core runs the full per-batch computation; no collectives.

Layouts (per core):
  - Activations are kept feature-major ("fm"): [feature_partitions, tokens],
    so every projection Y = X @ W becomes  Y_fm = W.T @ X_fm with W stored in
    its natural [in_feat, out_feat] orientation as the matmul lhsT.
  - v_c is computed token-major directly (lhsT = c_kv_fm) so attn@v needs no
    transpose.
  - Scores are computed transposed (scores_T[k_pos, q_pos]) so that
      E_T = exp(scores_T)  serves directly as the rhs of attn@v; row sums are
    computed off the PE: exact f32 tree-adds on DVE + a GpSimd
    partition_all_reduce (broadcasts the per-q sums to all partitions).
  - Softmax skips max-subtraction (scores are small: |s/scale| < ~3), exp is
    fused with the 1/scale into one ScalarE activation.
  - RoPE: the interleaved (even,odd) pairs are de-interleaved on the host by
    permuting Wqr/Wkr columns, so on-chip rotation is 6 tensor_tensor ops per
    256-row half-block. Dot products are invariant to the permutation since
    it is applied to both q_r and k_r.
  - Normalization (divide by softmax sum) is deferred: o_raw accumulates
    unnormalized; per 8 heads the sums are repacked [128, 32] via a DRAM
    bounce, one cheap packed DVE reciprocal, then broadcast back and applied
    as tensor_tensor * recip + b_uv (valid because sum_k attn = 1).
    (Reciprocal is ~6.3 cyc/elem on DVE, so it must be partition-packed;
    a per-head [128,512] reciprocal costs 3.4 us and stalls the PE.)

All matmul inputs are bf16 (fp32 accumulate in PSUM); final output fp32.
"""

import sys

import numpy as np
import ml_dtypes

for _p in ("/opt/trn_rl_repo",):
    if _p not in sys.path:
        sys.path.append(_p)

B, S, D, H = 8, 512, 2048, 16
DOWN, UP, R = 512, 2048, 512
VHD = UP // H          # 128
HD = D // H            # 128
SCALE = float(HD**0.5 + R**0.5)
P = 128
BF16 = ml_dtypes.bfloat16

_CACHE = {}


def _rope_tables_np():
    pos = np.arange(R, dtype=np.float32)
    div = np.exp(np.arange(0, R, 2, dtype=np.float32) * (-np.log(10000.0) / R))
    theta = np.outer(pos, div)          # [512, 256]
    return np.sin(theta), np.cos(theta)


def build_nc():
    """Build + compile the per-core Bass program. Returns (nc, input_names)."""
    import concourse.mybir as mybir
    import concourse.tile as tile
    from concourse import bacc
    from concourse import bass_isa

    f32 = mybir.dt.float32
    bf16 = mybir.dt.bfloat16
    Ident = mybir.ActivationFunctionType.Identity
    Exp = mybir.ActivationFunctionType.Exp
    MUL = mybir.AluOpType.mult
    ADD = mybir.AluOpType.add
    SUB = mybir.AluOpType.subtract

    nc = bacc.Bacc(
        "TRN2",
        target_bir_lowering=False,
        debug=False,
        enable_asserts=False,
        num_devices=8,
    )

    def din(name, shape, dt=bf16):
        return nc.dram_tensor(name, list(shape), dt, kind="ExternalInput").ap()

    # X^T and stage-1 weights in 4 chunks of 4 k-tiles each (earlier PE start).
    xt_d = [din(f"xt{c}", (P, 4, S)) for c in range(4)]
    wdq_d = [din(f"wdq{c}", (P, 4, DOWN)) for c in range(4)]
    wdkv_d = [din(f"wdkv{c}", (P, 4, DOWN)) for c in range(4)]
    wkr_d = [din(f"wkr{c}", (P, 4, R)) for c in range(4)]
    wuq_d = din("wuq", (H, P, 4, VHD))
    wuk_d = din("wuk", (H, P, 4, VHD))
    wuv_d = din("wuv", (P, 4, UP))           # [p, kt, head-group-major feats]
    wqr_d = din("wqr", (H, P, 4, R))
    wfc_d = din("wfc", (16, P, 16, P))
    cos_d = din("cosr", (P, 2, S))
    sin_d = din("sinr", (P, 2, S))
    bdq_d = din("bdq", (P, 4), f32)
    bdkv_d = din("bdkv", (P, 4), f32)
    bkr_d = din("bkr", (P, 4), f32)
    buq_d = din("buq", (P, H), f32)
    buk_d = din("buk", (P, H), f32)
    buv_d = din("buv", (P, H), f32)
    bqr_d = din("bqr", (P, 64), f32)
    bfc_d = din("bfc", (P, 16), f32)
    yt_d = nc.dram_tensor("yt", [D, S], f32, kind="ExternalOutput").ap()

    input_names = (
        [f"xt{c}" for c in range(4)]
        + [f"wdq{c}" for c in range(4)]
        + [f"wdkv{c}" for c in range(4)]
        + [f"wkr{c}" for c in range(4)]
        + ["wuq", "wuk", "wuv", "wqr", "wfc", "cosr", "sinr",
           "bdq", "bdkv", "bkr", "buq", "buk", "buv", "bqr", "bfc"]
    )

    with tile.TileContext(nc) as tc:
        with (
            tc.tile_pool(name="pconst", bufs=1) as pconst,
            tc.tile_pool(name="pbig", bufs=1) as pbig,
            tc.tile_pool(name="pwork", bufs=2) as pwork,
            tc.tile_pool(name="pps", bufs=8, space="PSUM") as pps,
            tc.tile_pool(name="pdram", bufs=1, space="DRAM") as pdram,
        ):
            # ---- stage-0 DMAs, ordered so the first matmuls start ASAP ----
            XT, WDQ, WDKV, WKR = [], [], [], []

            def load_chunk(lst, dram, nm, split=False):
                t = pbig.tile([P, 4, 512], bf16, tag=nm)
                if split:  # finer granularity so the first matmul starts ASAP
                    nc.sync.dma_start(t[:, 0:1, :], dram[:, 0:1, :])
                    nc.sync.dma_start(t[:, 1:4, :], dram[:, 1:4, :])
                else:
                    nc.sync.dma_start(t[:], dram[:])
                lst.append(t)

            load_chunk(XT, xt_d[0], "xt_0", split=True)
            load_chunk(WDQ, wdq_d[0], "w1_0", split=True)
            for c in range(1, 4):
                load_chunk(XT, xt_d[c], f"xt_{c}")
                load_chunk(WDQ, wdq_d[c], f"w1_{c}")
            bias = {}

            def load_bias(nm, ap_, k):
                t = pconst.tile([P, k], f32, tag=f"b_{nm}")
                nc.sync.dma_start(t[:], ap_[:])
                bias[nm] = t

            load_bias("bdq", bdq_d, 4)
            for c in range(4):
                load_chunk(WDKV, wdkv_d[c], f"w1_{4 + c}")
            load_bias("bdkv", bdkv_d, 4)
            for c in range(4):
                load_chunk(WKR, wkr_d[c], f"w1_{8 + c}")
            load_bias("bkr", bkr_d, 4)
            cos_t = pconst.tile([P, 2, S], bf16, tag="cos_t")
            sin_t = pconst.tile([P, 2, S], bf16, tag="sin_t")
            nc.sync.dma_start(cos_t[:], cos_d[:])
            nc.sync.dma_start(sin_t[:], sin_d[:])
            WUV = pbig.tile([P, 4, UP], bf16, tag="wuv")
            nc.sync.dma_start(WUV[:], wuv_d[:])
            for nm, ap_, k in (
                ("buq", buq_d, H), ("buk", buk_d, H), ("buv", buv_d, H),
                ("bqr", bqr_d, 64), ("bfc", bfc_d, 16),
            ):
                load_bias(nm, ap_, k)
            # persistent activations
            CQ = pbig.tile([P, 4, S], bf16, tag="cq")
            CKV = pbig.tile([P, 4, S], bf16, tag="ckv")
            KROT = pbig.tile([P, 4, S], bf16, tag="krot")
            VC = pbig.tile([P, 4, UP], bf16, tag="vc")
            ORAW = pbig.tile([P, H, S], bf16, tag="oraw")
            sums_dram = pdram.tile([H * S], f32)
            recip_dram = pdram.tile([H * S], bf16)

            def rope(dst, src, tmp_prefix):
                # src/dst: [P, 4, S] bf16; halves: tiles 0-1 = x1, 2-3 = x2
                for i in range(2):
                    x1 = src[:, i, :]
                    x2 = src[:, 2 + i, :]
                    c_ = cos_t[:, i, :]
                    s_ = sin_t[:, i, :]
                    t1 = pwork.tile([P, S], bf16, tag=f"{tmp_prefix}a")
                    t2 = pwork.tile([P, S], bf16, tag=f"{tmp_prefix}b")
                    nc.vector.tensor_tensor(t1[:], x1, c_, MUL)
                    nc.vector.tensor_tensor(t2[:], x2, s_, MUL)
                    nc.vector.tensor_tensor(dst[:, i, :], t1[:], t2[:], SUB)
                    t3 = pwork.tile([P, S], bf16, tag=f"{tmp_prefix}a")
                    t4 = pwork.tile([P, S], bf16, tag=f"{tmp_prefix}b")
                    nc.vector.tensor_tensor(t3[:], x2, c_, MUL)
                    nc.vector.tensor_tensor(t4[:], x1, s_, MUL)
                    nc.vector.tensor_tensor(dst[:, 2 + i, :], t3[:], t4[:], ADD)

            # ---- stage 1: c_q, c_kv, k_r(+rope) ----
            # kt-outer so matmuls start as soon as chunk 0 lands and stream
            # with the remaining chunk DMAs (4 psum accumulators at a time)
            KRAW = pbig.tile([P, 4, S], bf16, tag="kraw")
            for dst, W, b in (
                (CQ, WDQ, "bdq"), (CKV, WDKV, "bdkv"), (KRAW, WKR, "bkr"),
            ):
                pss = [
                    pps.tile([P, 512], f32, tag="ps", name=f"ps_s1_{i}")
                    for i in range(4)
                ]
                for kt in range(16):
                    for mt in range(4):
                        nc.tensor.matmul(
                            pss[mt][:],
                            W[kt // 4][:, kt % 4, mt * P:(mt + 1) * P],
                            XT[kt // 4][:, kt % 4, :],
                            start=(kt == 0),
                            stop=(kt == 15),
                        )
                for mt in range(4):
                    nc.scalar.activation(
                        dst[:, mt, :], pss[mt][:], Ident,
                        bias=bias[b][:, mt:mt + 1],
                    )
            rope(KROT, KRAW, "kr")

            # ---- stage 2: v_c token-major (no bias; folded into o-norm) ----
            for tt in range(4):
                for hg in range(4):
                    ps = pps.tile([P, 512], f32, tag="ps")
                    for kt in range(4):
                        nc.tensor.matmul(
                            ps[:],
                            CKV[:, kt, tt * P:(tt + 1) * P],
                            WUV[:, kt, hg * 512:(hg + 1) * 512],
                            start=(kt == 0),
                            stop=(kt == 3),
                        )
                    nc.any.tensor_copy(VC[:, tt, hg * 512:(hg + 1) * 512], ps[:])

            # ---- stage 3: per-head attention, software-pipelined ----
            # Phase A(h): weight DMAs + projections (q_r raw, rope, q_c, k_c)
            # Phase B(h): scores/exp, attn@v, sums
            # Emitted as A(0), A(1), B(0), A(2), B(1), ... so the PE stream of
            # A(h+1) covers B(h)'s wait on rope(h) (the engine streams are
            # statically ordered by the scheduler).
            qk_tiles = {}

            def phase_a(h):
                WQRh = pwork.tile([P, 4, R], bf16, tag="wqrh")
                nc.sync.dma_start(WQRh[:], wqr_d[h])
                WUQh = pwork.tile([P, 4, VHD], bf16, tag="wuqh")
                nc.sync.dma_start(WUQh[:], wuq_d[h])
                WUKh = pwork.tile([P, 4, VHD], bf16, tag="wukh")
                nc.sync.dma_start(WUKh[:], wuk_d[h])

                # q_r raw projection [R, S] fm
                QRAW = pwork.tile([P, 4, S], bf16, tag="qraw")
                for mt in range(4):
                    ps = pps.tile([P, 512], f32, tag="ps")
                    for kt in range(4):
                        nc.tensor.matmul(
                            ps[:],
                            WQRh[:, kt, mt * P:(mt + 1) * P],
                            CQ[:, kt, :],
                            start=(kt == 0),
                            stop=(kt == 3),
                        )
                    nc.scalar.activation(
                        QRAW[:, mt, :], ps[:], Ident,
                        bias=bias["bqr"][:, h * 4 + mt:h * 4 + mt + 1],
                    )
                # q_c, k_c [VHD, S] fm
                qc = pwork.tile([P, S], bf16, tag="qc")
                kc = pwork.tile([P, S], bf16, tag="kc")
                for dst, Wh, b, src in (
                    (qc, WUQh, "buq", CQ), (kc, WUKh, "buk", CKV),
                ):
                    ps = pps.tile([P, 512], f32, tag="ps")
                    for kt in range(4):
                        nc.tensor.matmul(
                            ps[:], Wh[:, kt, :], src[:, kt, :],
                            start=(kt == 0), stop=(kt == 3),
                        )
                    nc.scalar.activation(
                        dst[:], ps[:], Ident, bias=bias[b][:, h:h + 1]
                    )
                QROT = pwork.tile([P, 4, S], bf16, tag="qrot")
                rope(QROT, QRAW, "qr")
                qk_tiles[h] = (qc, kc, QROT)

            def phase_b(h):
                qc, kc, QROT = qk_tiles.pop(h)
                # scores_T [k_pos, q_pos] -> E_T = exp(s/SCALE), bf16
                E = pwork.tile([P, 4, S], bf16, tag="E")
                for kp in range(4):
                    ps = pps.tile([P, 512], f32, tag="ps")
                    pieces = [(kc[:, kp * P:(kp + 1) * P], qc[:])]
                    pieces += [
                        (KROT[:, f, kp * P:(kp + 1) * P], QROT[:, f, :])
                        for f in range(4)
                    ]
                    for i, (lhsT, rhs) in enumerate(pieces):
                        nc.tensor.matmul(
                            ps[:], lhsT, rhs, start=(i == 0), stop=(i == 4)
                        )
                    nc.scalar.activation(
                        E[:, kp, :], ps[:], Exp, scale=1.0 / SCALE
                    )

                # o_raw^T [VHD, S] fm (unnormalized); fast evac frees the bank
                ps_o = pps.tile([P, 512], f32, tag="ps")
                for kp in range(4):
                    nc.tensor.matmul(
                        ps_o[:],
                        VC[:, kp, h * VHD:(h + 1) * VHD],
                        E[:, kp, :],
                        start=(kp == 0),
                        stop=(kp == 3),
                    )
                nc.any.tensor_copy(ORAW[:, h, :], ps_o[:])

                # softmax sums off the PE: exact f32 pairwise adds of the E
                # tiles on DVE, then a GpSimd partition all-reduce; row 0 of
                # the broadcast result is staged into a flat per-half buffer
                # for the batched (partition-packed) reciprocal below.
                e01 = pwork.tile([P, S], f32, tag="e01")
                e23 = pwork.tile([P, S], f32, tag="e23")
                nc.vector.tensor_tensor(e01[:], E[:, 0, :], E[:, 1, :], ADD)
                nc.vector.tensor_tensor(e23[:], E[:, 2, :], E[:, 3, :], ADD)
                nc.vector.tensor_tensor(e01[:], e01[:], e23[:], ADD)
                sall = pwork.tile([P, S], f32, tag="sall")
                nc.gpsimd.partition_all_reduce(
                    sall[:], e01[:], channels=P, reduce_op=bass_isa.ReduceOp.add
                )
                nc.sync.dma_start(sums_dram[h * S:(h + 1) * S][None], sall[0:1, :])

                # every 8 heads: packed recip + normalize
                NB = 8
                if h % NB == NB - 1:
                    hb = h // NB
                    sl = slice(hb * NB * S, (hb + 1) * NB * S)
                    SUMS2 = pwork.tile([P, NB * S // P], f32, tag="sums2")
                    nc.sync.dma_start(
                        SUMS2[:], sums_dram[sl].rearrange("(p j) -> p j", p=P)
                    )
                    RECIP2 = pwork.tile([P, NB * S // P], bf16, tag="recip2")
                    with nc.allow_low_precision(reason="softmax denom recip"):
                        nc.vector.reciprocal(RECIP2[:], SUMS2[:])
                    nc.sync.dma_start(
                        recip_dram[sl].rearrange("(p j) -> p j", p=P), RECIP2[:]
                    )
                    for hh in range(hb * NB, (hb + 1) * NB):
                        rb = pwork.tile([P, S], bf16, tag="rb")
                        nc.sync.dma_start(
                            rb[:],
                            recip_dram[hh * S:(hh + 1) * S][None]
                            .to_broadcast((P, S)),
                        )
                        nc.vector.tensor_tensor(
                            ORAW[:, hh, :], ORAW[:, hh, :], rb[:], MUL
                        )
                        nc.vector.tensor_scalar_add(
                            ORAW[:, hh, :], ORAW[:, hh, :], bias["buv"][:, hh:hh + 1]
                        )

            phase_a(0)
            for h in range(16):
                if h + 1 < 16:
                    phase_a(h + 1)
                phase_b(h)

            # ---- stage 5: fc ----
            for mt in range(16):
                WFCt = pbig.tile([P, 16, P], bf16, tag=f"w1_{mt % 12}")
                nc.sync.dma_start(WFCt[:], wfc_d[mt])
                ps = pps.tile([P, 512], f32, tag="ps")
                for kt in range(16):
                    nc.tensor.matmul(
                        ps[:], WFCt[:, kt, :], ORAW[:, kt, :],
                        start=(kt == 0), stop=(kt == 15),
                    )
                y = pwork.tile([P, 512], f32, tag="y")
                nc.scalar.activation(
                    y[:], ps[:], Ident, bias=bias["bfc"][:, mt:mt + 1]
                )
                nc.sync.dma_start(yt_d[mt * P:(mt + 1) * P, :], y[:])

    nc.compile()
    return nc, input_names


def prepare_in_maps(inputs):
    """Host-side prep: cast to bf16, de-interleave rope dims, tile layouts."""
    g = {k: np.asarray(v, dtype=np.float32) for k, v in inputs.items()}
    perm = np.concatenate([np.arange(0, R, 2), np.arange(1, R, 2)])

    def chunks16(w):  # [2048, M] -> 4 chunks [128, 4, M]
        kt = w.reshape(16, P, w.shape[1])
        return [
            np.ascontiguousarray(kt[4 * c:4 * c + 4].transpose(1, 0, 2)).astype(BF16)
            for c in range(4)
        ]

    def bcol(b, k):  # [k*128] -> [128, k] fp32
        return np.ascontiguousarray(b.reshape(k, P).T)

    wqr_p = g["Wqr"].reshape(DOWN, H, R)[:, :, perm]
    wkr_p = g["Wkr"][:, perm]
    bqr_p = g["bqr"].reshape(H, R)[:, perm]
    bkr_p = g["bkr"][perm]

    common = {}
    for c, (a, b_, d) in enumerate(
        zip(chunks16(g["Wdq"]), chunks16(g["Wdkv"]), chunks16(wkr_p))
    ):
        common[f"wdq{c}"] = a
        common[f"wdkv{c}"] = b_
        common[f"wkr{c}"] = d
    common["wuq"] = np.ascontiguousarray(
        g["Wuq"].reshape(4, P, H, VHD).transpose(2, 1, 0, 3)).astype(BF16)
    common["wuk"] = np.ascontiguousarray(
        g["Wuk"].reshape(4, P, H, VHD).transpose(2, 1, 0, 3)).astype(BF16)
    common["wuv"] = np.ascontiguousarray(
        g["Wuv"].reshape(4, P, UP).transpose(1, 0, 2)).astype(BF16)
    common["wqr"] = np.ascontiguousarray(
        wqr_p.transpose(1, 0, 2).reshape(H, 4, P, R).transpose(0, 2, 1, 3)
    ).astype(BF16)
    common["wfc"] = np.ascontiguousarray(
        g["Wfc"].reshape(16, P, 16, P).transpose(2, 1, 0, 3)).astype(BF16)

    sin_t, cos_t = _rope_tables_np()      # [512, 256]
    for nm, t in (("cosr", cos_t), ("sinr", sin_t)):
        common[nm] = np.ascontiguousarray(
            t.T.reshape(2, P, S).transpose(1, 0, 2)).astype(BF16)

    common["bdq"] = bcol(g["bdq"], 4)
    common["bdkv"] = bcol(g["bdkv"], 4)
    common["bkr"] = bcol(bkr_p, 4)
    common["buq"] = bcol(g["buq"], H)
    common["buk"] = bcol(g["buk"], H)
    common["buv"] = bcol(g["buv"], H)
    common["bqr"] = bcol(bqr_p.reshape(-1), 64)
    common["bfc"] = bcol(g["bfc"], 16)

    in_maps = []
    for b in range(B):
        m = dict(common)
        xtb = g["X"][b].T.astype(BF16)        # [2048, 512]
        kt = xtb.reshape(16, P, S)
        for c in range(4):
            m[f"xt{c}"] = np.ascontiguousarray(
                kt[4 * c:4 * c + 4].transpose(1, 0, 2))
        in_maps.append(m)
    return in_maps


def _get_program():
    if "nc" not in _CACHE:
        _CACHE["nc"], _CACHE["input_names"] = build_nc()
    return _CACHE["nc"], _CACHE["input_names"]


def kernel(**inputs) -> np.ndarray:
    from concourse.bass_utils import run_bass_kernel_spmd

    nc, _ = _get_program()
    in_maps = prepare_in_maps(inputs)
    res = run_bass_kernel_spmd(nc, in_maps, core_ids=list(range(B)))
    out = np.stack(
        [np.ascontiguousarray(res.results[b]["yt"].T) for b in range(B)]
    )
    return out.astype(np.float32)



# revision 29
# speedup vs baseline: 1.0153x; 1.0153x over previous
"""MLA (multi-head latent attention) Bass kernel for Trainium2, 8 NeuronCores.

Sharding: pure data-parallel over batch (B=8 -> one batch element per core).
Each core runs the full per-batch computation; no collectives.

Layouts (per core):
  - Activations are kept feature-major ("fm"): [feature_partitions, tokens],
    so every projection Y = X @ W becomes  Y_fm = W.T @ X_fm with W stored in
    its natural [in_feat, out_feat] orientation as the matmul lhsT.
  - v_c is computed token-major directly (lhsT = c_kv_fm) so attn@v needs no
    transpose.
  - Scores are computed transposed (scores_T[k_pos, q_pos]) so that
      E_T = exp(scores_T)  serves directly as the rhs of attn@v; row sums are
    computed off the PE: exact f32 tree-adds on DVE + a GpSimd
    partition_all_reduce (broadcasts the per-q sums to all partitions).
  - Softmax skips max-subtraction (scores are small: |s/scale| < ~3), exp is
    fused with the 1/scale into one ScalarE activation.
  - RoPE: the interleaved (even,odd) pairs are de-interleaved on the host by
    permuting Wqr/Wkr columns, so on-chip rotation is 6 tensor_tensor ops per
    256-row half-block. Dot products are invariant to the permutation since
    it is applied to both q_r and k_r.
  - Normalization (divide by softmax sum) is deferred: o_raw accumulates
    unnormalized; per 8 heads the sums are repacked [128, 32] via a DRAM
    bounce, one cheap packed DVE reciprocal, then broadcast back and applied
    as tensor_tensor * recip + b_uv (valid because sum_k attn = 1).
    (Reciprocal is ~6.3 cyc/elem on DVE, so it must be partition-packed;
    a per-head [128,512] reciprocal costs 3.4 us and stalls the PE.)

All matmul inputs are bf16 (fp32 accumulate in PSUM); final output fp32.
"""

import sys

import numpy as np
import ml_dtypes

for _p in ("/opt/trn_rl_repo",):
    if _p not in sys.path:
        sys.path.append(_p)

B, S, D, H = 8, 512, 2048, 16
DOWN, UP, R = 512, 2048, 512
VHD = UP // H          # 128
HD = D // H            # 128
SCALE = float(HD**0.5 + R**0.5)
P = 128
BF16 = ml_dtypes.bfloat16

_CACHE = {}


def _rope_tables_np():
    pos = np.arange(R, dtype=np.float32)
    div = np.exp(np.arange(0, R, 2, dtype=np.float32) * (-np.log(10000.0) / R))
    theta = np.outer(pos, div)          # [512, 256]
    return np.sin(theta), np.cos(theta)


def build_nc():
    """Build + compile the per-core Bass program. Returns (nc, input_names)."""
    import concourse.mybir as mybir
    import concourse.tile as tile
    from concourse import bacc
    from concourse import bass_isa

    f32 = mybir.dt.float32
    bf16 = mybir.dt.bfloat16
    fp8 = mybir.dt.float8e4
    DR = mybir.MatmulPerfMode.DoubleRow
    Ident = mybir.ActivationFunctionType.Identity
    Exp = mybir.ActivationFunctionType.Exp
    MUL = mybir.AluOpType.mult
    ADD = mybir.AluOpType.add
    SUB = mybir.AluOpType.subtract

    nc = bacc.Bacc(
        "TRN2",
        target_bir_lowering=False,
        debug=False,
        enable_asserts=False,
        num_devices=8,
    )

    def din(name, shape, dt=bf16):
        return nc.dram_tensor(name, list(shape), dt, kind="ExternalInput").ap()

    # X^T and stage-1 weights in 4 chunks of 4 k-tiles each (earlier PE start).
    xt_d = [din(f"xt{c}", (P, 4, S)) for c in range(4)]
    wdq_d = [din(f"wdq{c}", (P, 4, DOWN)) for c in range(4)]
    wdkv_d = [din(f"wdkv{c}", (P, 4, DOWN)) for c in range(4)]
    wkr_d = [din(f"wkr{c}", (P, 4, R)) for c in range(4)]
    wuq_d = din("wuq", (H, P, 4, VHD))
    wuk_d = din("wuk", (H, P, 4, VHD))
    wuv_d = din("wuv", (P, 4, UP))           # [p, kt, head-group-major feats]
    wqr_d = din("wqr", (H, P, 4, R), fp8)
    wfc_d = din("wfc", (16, P, 16, P))
    cos_d = din("cosr", (P, 2, S))
    sin_d = din("sinr", (P, 2, S))
    bdq_d = din("bdq", (P, 4), f32)
    bdkv_d = din("bdkv", (P, 4), f32)
    bkr_d = din("bkr", (P, 4), f32)
    buq_d = din("buq", (P, H), f32)
    buk_d = din("buk", (P, H), f32)
    buv_d = din("buv", (P, H), f32)
    bqr_d = din("bqr", (P, 64), f32)
    bfc_d = din("bfc", (P, 16), f32)
    yt_d = nc.dram_tensor("yt", [D, S], f32, kind="ExternalOutput").ap()

    input_names = (
        [f"xt{c}" for c in range(4)]
        + [f"wdq{c}" for c in range(4)]
        + [f"wdkv{c}" for c in range(4)]
        + [f"wkr{c}" for c in range(4)]
        + ["wuq", "wuk", "wuv", "wqr", "wfc", "cosr", "sinr",
           "bdq", "bdkv", "bkr", "buq", "buk", "buv", "bqr", "bfc"]
    )

    with tile.TileContext(nc) as tc:
        with (
            tc.tile_pool(name="pconst", bufs=1) as pconst,
            tc.tile_pool(name="pbig", bufs=1) as pbig,
            tc.tile_pool(name="pwork", bufs=2) as pwork,
            tc.tile_pool(name="pps", bufs=8, space="PSUM") as pps,
            tc.tile_pool(name="pdram", bufs=1, space="DRAM") as pdram,
        ):
            # ---- stage-0 DMAs, ordered so the first matmuls start ASAP ----
            XT, WDQ, WDKV, WKR = [], [], [], []

            def load_chunk(lst, dram, nm, split=False):
                t = pbig.tile([P, 4, 512], bf16, tag=nm)
                if split:  # finer granularity so the first matmul starts ASAP
                    nc.sync.dma_start(t[:, 0:1, :], dram[:, 0:1, :])
                    nc.sync.dma_start(t[:, 1:4, :], dram[:, 1:4, :])
                else:
                    nc.sync.dma_start(t[:], dram[:])
                lst.append(t)

            load_chunk(XT, xt_d[0], "xt_0", split=True)
            load_chunk(WDQ, wdq_d[0], "w1_0", split=True)
            for c in range(1, 4):
                load_chunk(XT, xt_d[c], f"xt_{c}")
                load_chunk(WDQ, wdq_d[c], f"w1_{c}")
            bias = {}

            def load_bias(nm, ap_, k):
                t = pconst.tile([P, k], f32, tag=f"b_{nm}")
                nc.sync.dma_start(t[:], ap_[:])
                bias[nm] = t

            load_bias("bdq", bdq_d, 4)
            for c in range(4):
                load_chunk(WDKV, wdkv_d[c], f"w1_{4 + c}")
            load_bias("bdkv", bdkv_d, 4)
            for c in range(4):
                load_chunk(WKR, wkr_d[c], f"w1_{8 + c}")
            load_bias("bkr", bkr_d, 4)
            cos_t = pconst.tile([P, 2, S], bf16, tag="cos_t")
            sin_t = pconst.tile([P, 2, S], bf16, tag="sin_t")
            nc.sync.dma_start(cos_t[:], cos_d[:])
            nc.sync.dma_start(sin_t[:], sin_d[:])
            WUV = pbig.tile([P, 4, UP], bf16, tag="wuv")
            nc.sync.dma_start(WUV[:], wuv_d[:])
            for nm, ap_, k in (
                ("buq", buq_d, H), ("buk", buk_d, H), ("buv", buv_d, H),
                ("bqr", bqr_d, 64), ("bfc", bfc_d, 16),
            ):
                load_bias(nm, ap_, k)
            # persistent activations
            CQ = pbig.tile([P, 4, S], bf16, tag="cq")
            CKV = pbig.tile([P, 4, S], bf16, tag="ckv")
            KROT = pbig.tile([P, 4, S], fp8, tag="krot")
            VC = pbig.tile([P, 4, UP], bf16, tag="vc")
            ORAW = pbig.tile([P, H, S], bf16, tag="oraw")
            sums_dram = pdram.tile([H * S], f32)
            recip_dram = pdram.tile([H * S], bf16)

            def rope(dst, src, tmp_prefix):
                # src/dst: [P, 4, S] bf16; halves: tiles 0-1 = x1, 2-3 = x2
                for i in range(2):
                    x1 = src[:, i, :]
                    x2 = src[:, 2 + i, :]
                    c_ = cos_t[:, i, :]
                    s_ = sin_t[:, i, :]
                    t1 = pwork.tile([P, S], bf16, tag=f"{tmp_prefix}a")
                    t2 = pwork.tile([P, S], bf16, tag=f"{tmp_prefix}b")
                    nc.vector.tensor_tensor(t1[:], x1, c_, MUL)
                    nc.vector.tensor_tensor(t2[:], x2, s_, MUL)
                    nc.vector.tensor_tensor(dst[:, i, :], t1[:], t2[:], SUB)
                    t3 = pwork.tile([P, S], bf16, tag=f"{tmp_prefix}a")
                    t4 = pwork.tile([P, S], bf16, tag=f"{tmp_prefix}b")
                    nc.vector.tensor_tensor(t3[:], x2, c_, MUL)
                    nc.vector.tensor_tensor(t4[:], x1, s_, MUL)
                    nc.vector.tensor_tensor(dst[:, 2 + i, :], t3[:], t4[:], ADD)

            # ---- stage 1: c_q, c_kv, k_r(+rope) ----
            # kt-outer so matmuls start as soon as chunk 0 lands and stream
            # with the remaining chunk DMAs (4 psum accumulators at a time)
            KRAW = pbig.tile([P, 4, S], bf16, tag="kraw")
            for dst, W, b in (
                (CQ, WDQ, "bdq"), (CKV, WDKV, "bdkv"), (KRAW, WKR, "bkr"),
            ):
                pss = [
                    pps.tile([P, 512], f32, tag="ps", name=f"ps_s1_{i}")
                    for i in range(4)
                ]
                for kt in range(16):
                    for mt in range(4):
                        nc.tensor.matmul(
                            pss[mt][:],
                            W[kt // 4][:, kt % 4, mt * P:(mt + 1) * P],
                            XT[kt // 4][:, kt % 4, :],
                            start=(kt == 0),
                            stop=(kt == 15),
                        )
                for mt in range(4):
                    nc.scalar.activation(
                        dst[:, mt, :], pss[mt][:], Ident,
                        bias=bias[b][:, mt:mt + 1],
                    )
            CQ8 = pbig.tile([P, 4, S], fp8, tag="cq8")
            for mt in range(4):
                nc.vector.tensor_copy(CQ8[:, mt, :], CQ[:, mt, :])
            rope(KROT, KRAW, "kr")

            # ---- stage 2: v_c token-major (no bias; folded into o-norm) ----
            for tt in range(4):
                for hg in range(4):
                    ps = pps.tile([P, 512], f32, tag="ps")
                    for kt in range(4):
                        nc.tensor.matmul(
                            ps[:],
                            CKV[:, kt, tt * P:(tt + 1) * P],
                            WUV[:, kt, hg * 512:(hg + 1) * 512],
                            start=(kt == 0),
                            stop=(kt == 3),
                        )
                    nc.any.tensor_copy(VC[:, tt, hg * 512:(hg + 1) * 512], ps[:])

            # ---- stage 3: per-head attention, software-pipelined ----
            # Phase A(h): weight DMAs + projections (q_r raw, rope, q_c, k_c)
            # Phase B(h): scores/exp, attn@v, sums
            # Emitted as A(0), A(1), B(0), A(2), B(1), ... so the PE stream of
            # A(h+1) covers B(h)'s wait on rope(h) (the engine streams are
            # statically ordered by the scheduler).
            qk_tiles = {}

            def phase_a(h):
                WQRh = pwork.tile([P, 4, R], fp8, tag="wqrh")
                nc.sync.dma_start(WQRh[:], wqr_d[h])
                WUQh = pwork.tile([P, 4, VHD], bf16, tag="wuqh")
                nc.sync.dma_start(WUQh[:], wuq_d[h])
                WUKh = pwork.tile([P, 4, VHD], bf16, tag="wukh")
                nc.sync.dma_start(WUKh[:], wuk_d[h])

                # q_r raw projection [R, S] fm
                QRAW = pwork.tile([P, 4, S], bf16, tag="qraw")
                for mt in range(4):
                    ps = pps.tile([P, 512], f32, tag="ps")
                    for kt in (0, 2):
                        nc.tensor.matmul(
                            ps[:],
                            WQRh[:, kt:kt + 2, mt * P:(mt + 1) * P],
                            CQ8[:, kt:kt + 2, :],
                            start=(kt == 0),
                            stop=(kt == 2),
                            perf_mode=DR,
                        )
                    nc.scalar.activation(
                        QRAW[:, mt, :], ps[:], Ident,
                        bias=bias["bqr"][:, h * 4 + mt:h * 4 + mt + 1],
                    )
                # q_c, k_c [VHD, S] fm
                qc = pwork.tile([P, S], bf16, tag="qc")
                kc = pwork.tile([P, S], bf16, tag="kc")
                for dst, Wh, b, src in (
                    (qc, WUQh, "buq", CQ), (kc, WUKh, "buk", CKV),
                ):
                    ps = pps.tile([P, 512], f32, tag="ps")
                    for kt in range(4):
                        nc.tensor.matmul(
                            ps[:], Wh[:, kt, :], src[:, kt, :],
                            start=(kt == 0), stop=(kt == 3),
                        )
                    nc.scalar.activation(
                        dst[:], ps[:], Ident, bias=bias[b][:, h:h + 1]
                    )
                QROT = pwork.tile([P, 4, S], fp8, tag="qrot")
                rope(QROT, QRAW, "qr")
                qk_tiles[h] = (qc, kc, QROT)

            def phase_b(h):
                qc, kc, QROT = qk_tiles.pop(h)
                # scores_T [k_pos, q_pos] -> E_T = exp(s/SCALE), bf16
                E = pwork.tile([P, 4, S], bf16, tag="E")
                for kp in range(4):
                    ps = pps.tile([P, 512], f32, tag="ps")
                    nc.tensor.matmul(
                        ps[:], kc[:, kp * P:(kp + 1) * P], qc[:],
                        start=True, stop=False,
                    )
                    for f in (0, 2):
                        nc.tensor.matmul(
                            ps[:],
                            KROT[:, f:f + 2, kp * P:(kp + 1) * P],
                            QROT[:, f:f + 2, :],
                            start=False,
                            stop=(f == 2),
                            perf_mode=DR,
                        )
                    nc.scalar.activation(
                        E[:, kp, :], ps[:], Exp, scale=1.0 / SCALE
                    )

                # o_raw^T [VHD, S] fm (unnormalized); fast evac frees the bank
                ps_o = pps.tile([P, 512], f32, tag="ps")
                for kp in range(4):
                    nc.tensor.matmul(
                        ps_o[:],
                        VC[:, kp, h * VHD:(h + 1) * VHD],
                        E[:, kp, :],
                        start=(kp == 0),
                        stop=(kp == 3),
                    )
                nc.any.tensor_copy(ORAW[:, h, :], ps_o[:])

                # softmax sums off the PE: exact f32 pairwise adds of the E
                # tiles on DVE, then a GpSimd partition all-reduce; row 0 of
                # the broadcast result is staged into a flat per-half buffer
                # for the batched (partition-packed) reciprocal below.
                e01 = pwork.tile([P, S], f32, tag="e01")
                e23 = pwork.tile([P, S], f32, tag="e23")
                nc.vector.tensor_tensor(e01[:], E[:, 0, :], E[:, 1, :], ADD)
                nc.vector.tensor_tensor(e23[:], E[:, 2, :], E[:, 3, :], ADD)
                nc.vector.tensor_tensor(e01[:], e01[:], e23[:], ADD)
                sall = pwork.tile([P, S], f32, tag="sall")
                nc.gpsimd.partition_all_reduce(
                    sall[:], e01[:], channels=P, reduce_op=bass_isa.ReduceOp.add
                )
                nc.sync.dma_start(sums_dram[h * S:(h + 1) * S][None], sall[0:1, :])

                # every 8 heads: packed recip + normalize
                NB = 8
                if h % NB == NB - 1:
                    hb = h // NB
                    sl = slice(hb * NB * S, (hb + 1) * NB * S)
                    SUMS2 = pwork.tile([P, NB * S // P], f32, tag="sums2")
                    nc.sync.dma_start(
                        SUMS2[:], sums_dram[sl].rearrange("(p j) -> p j", p=P)
                    )
                    RECIP2 = pwork.tile([P, NB * S // P], bf16, tag="recip2")
                    with nc.allow_low_precision(reason="softmax denom recip"):
                        nc.vector.reciprocal(RECIP2[:], SUMS2[:])
                    nc.sync.dma_start(
                        recip_dram[sl].rearrange("(p j) -> p j", p=P), RECIP2[:]
                    )
                    for hh in range(hb * NB, (hb + 1) * NB):
                        rb = pwork.tile([P, S], bf16, tag="rb")
                        nc.sync.dma_start(
                            rb[:],
                            recip_dram[hh * S:(hh + 1) * S][None]
                            .to_broadcast((P, S)),
                        )
                        nc.vector.tensor_tensor(
                            ORAW[:, hh, :], ORAW[:, hh, :], rb[:], MUL
                        )
                        nc.vector.tensor_scalar_add(
                            ORAW[:, hh, :], ORAW[:, hh, :], bias["buv"][:, hh:hh + 1]
                        )

            phase_a(0)
            for h in range(16):
                if h + 1 < 16:
                    phase_a(h + 1)
                phase_b(h)

            # ---- stage 5: fc ----
            for mt in range(16):
                WFCt = pbig.tile([P, 16, P], bf16, tag=f"w1_{mt % 12}")
                nc.sync.dma_start(WFCt[:], wfc_d[mt])
                ps = pps.tile([P, 512], f32, tag="ps")
                for kt in range(16):
                    nc.tensor.matmul(
                        ps[:], WFCt[:, kt, :], ORAW[:, kt, :],
                        start=(kt == 0), stop=(kt == 15),
                    )
                y = pwork.tile([P, 512], f32, tag="y")
                nc.scalar.activation(
                    y[:], ps[:], Ident, bias=bias["bfc"][:, mt:mt + 1]
                )
                nc.sync.dma_start(yt_d[mt * P:(mt + 1) * P, :], y[:])

    nc.compile()
    return nc, input_names


def prepare_in_maps(inputs):
    """Host-side prep: cast to bf16, de-interleave rope dims, tile layouts."""
    g = {k: np.asarray(v, dtype=np.float32) for k, v in inputs.items()}
    perm = np.concatenate([np.arange(0, R, 2), np.arange(1, R, 2)])

    def chunks16(w):  # [2048, M] -> 4 chunks [128, 4, M]
        kt = w.reshape(16, P, w.shape[1])
        return [
            np.ascontiguousarray(kt[4 * c:4 * c + 4].transpose(1, 0, 2)).astype(BF16)
            for c in range(4)
        ]

    def bcol(b, k):  # [k*128] -> [128, k] fp32
        return np.ascontiguousarray(b.reshape(k, P).T)

    wqr_p = g["Wqr"].reshape(DOWN, H, R)[:, :, perm]
    wkr_p = g["Wkr"][:, perm]
    bqr_p = g["bqr"].reshape(H, R)[:, perm]
    bkr_p = g["bkr"][perm]

    common = {}
    for c, (a, b_, d) in enumerate(
        zip(chunks16(g["Wdq"]), chunks16(g["Wdkv"]), chunks16(wkr_p))
    ):
        common[f"wdq{c}"] = a
        common[f"wdkv{c}"] = b_
        common[f"wkr{c}"] = d
    common["wuq"] = np.ascontiguousarray(
        g["Wuq"].reshape(4, P, H, VHD).transpose(2, 1, 0, 3)).astype(BF16)
    common["wuk"] = np.ascontiguousarray(
        g["Wuk"].reshape(4, P, H, VHD).transpose(2, 1, 0, 3)).astype(BF16)
    common["wuv"] = np.ascontiguousarray(
        g["Wuv"].reshape(4, P, UP).transpose(1, 0, 2)).astype(BF16)
    common["wqr"] = np.ascontiguousarray(
        wqr_p.transpose(1, 0, 2).reshape(H, 4, P, R).transpose(0, 2, 1, 3)
    ).astype(ml_dtypes.float8_e4m3)
    common["wfc"] = np.ascontiguousarray(
        g["Wfc"].reshape(16, P, 16, P).transpose(2, 1, 0, 3)).astype(BF16)

    sin_t, cos_t = _rope_tables_np()      # [512, 256]
    for nm, t in (("cosr", cos_t), ("sinr", sin_t)):
        common[nm] = np.ascontiguousarray(
            t.T.reshape(2, P, S).transpose(1, 0, 2)).astype(BF16)

    common["bdq"] = bcol(g["bdq"], 4)
    common["bdkv"] = bcol(g["bdkv"], 4)
    common["bkr"] = bcol(bkr_p, 4)
    common["buq"] = bcol(g["buq"], H)
    common["buk"] = bcol(g["buk"], H)
    common["buv"] = bcol(g["buv"], H)
    common["bqr"] = bcol(bqr_p.reshape(-1), 64)
    common["bfc"] = bcol(g["bfc"], 16)

    in_maps = []
    for b in range(B):
        m = dict(common)
        xtb = g["X"][b].T.astype(BF16)        # [2048, 512]
        kt = xtb.reshape(16, P, S)
        for c in range(4):
            m[f"xt{c}"] = np.ascontiguousarray(
                kt[4 * c:4 * c + 4].transpose(1, 0, 2))
        in_maps.append(m)
    return in_maps


def _get_program():
    if "nc" not in _CACHE:
        _CACHE["nc"], _CACHE["input_names"] = build_nc()
    return _CACHE["nc"], _CACHE["input_names"]


def kernel(**inputs) -> np.ndarray:
    from concourse.bass_utils import run_bass_kernel_spmd

    nc, _ = _get_program()
    in_maps = prepare_in_maps(inputs)
    res = run_bass_kernel_spmd(nc, in_maps, core_ids=list(range(B)))
    out = np.stack(
        [np.ascontiguousarray(res.results[b]["yt"].T) for b in range(B)]
    )
    return out.astype(np.float32)

